# revision 44
# baseline (speedup 1.0000x reference)
"""Bass/Trainium2 kernel for DenseNetWithPCAM (pooling head + 2-layer MLP).

Computation (reference):
    ch_max = max(features, axis=(2,3))            # [B, C]
    pcams  = features * ch_max[..., None, None]   # [B, C, H, W]
    h      = relu(pcams.reshape(B, -1) @ W1.T + b1)
    logits = h @ W2.T + b2
    return (logits, features)

Sharding (8 NeuronCores, tensor-parallel over the contraction dim):
    Core c owns channels [128c, 128(c+1)) -> flat cols [6272c, 6272(c+1)).
    Per core: PCAM on-chip (reduce_max + broadcast multiply, bf16),
    PE-transpose pcams into [f, b] tiles, then matmul with the W1 shard
    streamed as the moving operand -> partial h [256 b, 768 d] in natural
    orientation (b1 folded in as a ones-row on core 0 only).  One bf16
    ReduceScatter over the batch dim gives each core 32 fully-reduced rows;
    relu + W2 matmul produce a [32, 5] logits shard per core and the host
    concatenates the 8 shards.  Features pass through on the host.
"""

import sys
import types

import numpy as np
import ml_dtypes

import concourse.bass as bass
import concourse.mybir as mybir
import concourse.tile as tile
from concourse import bacc
from concourse.bass_utils import run_bass_kernel_spmd

# bass_utils' BASS_TRACE path imports antenv.axon_hooks, which this
# container's antenv stub does not ship; registering a no-op hook keeps a
# trace-enabled environment from crashing (it just skips NTFF capture).
try:
    import antenv.axon_hooks  # noqa: F401
except ImportError:
    _m = types.ModuleType("antenv.axon_hooks")
    _m.get_axon_ntff_profile_hook = lambda: None
    sys.modules["antenv.axon_hooks"] = _m

# Problem shape (hardcoded per contract)
B, C, H, W_SP = 256, 1024, 7, 7
HW = H * W_SP                    # 49
D_HID, N_CLS = 768, 5
N_CORES = 8
C_PER = C // N_CORES             # 128 channels per core
F_PER = C_PER * HW               # 6272 flat features per core
K_TILES = F_PER // 128           # 49 contraction tiles of 128
D_TILES = D_HID // 128           # 6
B_SH = B // N_CORES              # 32 batch rows per core after RS
BF16 = mybir.dt.bfloat16
F32 = mybir.dt.float32

_CACHE = {}


def _build_program():
    nc = bacc.Bacc("TRN2", target_bir_lowering=False, debug=False,
                   num_devices=N_CORES)

    featN = nc.dram_tensor("featN", [B, F_PER], BF16, kind="ExternalInput")
    w1t = nc.dram_tensor("w1t", [F_PER, D_HID], BF16, kind="ExternalInput")
    b1row = nc.dram_tensor("b1row", [1, D_HID], BF16, kind="ExternalInput")
    w2t = nc.dram_tensor("w2t", [D_HID, N_CLS], BF16, kind="ExternalInput")
    b2b = nc.dram_tensor("b2b", [B_SH, N_CLS], F32, kind="ExternalInput")
    ident = nc.dram_tensor("ident", [128, 128], BF16, kind="ExternalInput")
    logits_sh = nc.dram_tensor("logits_sh", [B_SH, N_CLS], F32,
                               kind="ExternalOutput")

    with tile.TileContext(nc) as tc:
        with (
            tc.tile_pool(name="const", bufs=1) as constp,
            tc.tile_pool(name="feat", bufs=2) as featp,
            tc.tile_pool(name="cmax", bufs=2) as cmaxp,
            tc.tile_pool(name="pcam", bufs=2) as pcamp,
            tc.tile_pool(name="w1", bufs=12) as w1p,
            tc.tile_pool(name="at", bufs=8) as atp,
            tc.tile_pool(name="hsb", bufs=1) as hsbp,
            tc.tile_pool(name="psH", bufs=2, space="PSUM") as psH,
            tc.tile_pool(name="psB", bufs=4, space="PSUM") as psB,
            tc.tile_pool(name="dram", bufs=1, space="DRAM") as dramp,
        ):
            # ---- constants (only ident is on the critical path) ----
            id_sb = constp.tile([128, 128], BF16, tag="ident")
            nc.sync.dma_start(id_sb[:], ident[:])

            # ---- stage A: load features, per-channel max, pcam multiply ----
            # Chunked over channels so the DVE/GpSimd work pipelines under
            # the PE stage (Tile tracks subtile deps).
            CHUNKS = [16, 16, 32, 64]
            ft_tiles, cm_tiles, pcam_tiles = [], [], []
            for bt in range(2):
                ft_tiles.append(featp.tile([128, F_PER], BF16, tag="feat",
                                           name=f"ft{bt}"))
                cm_tiles.append(cmaxp.tile([128, C_PER], BF16, tag="cmax",
                                           name=f"cm{bt}"))
                pcam_tiles.append(pcamp.tile([128, F_PER], BF16, tag="pcam",
                                             name=f"pc{bt}"))
            c_off = 0
            for cch in CHUNKS:
                fsl = slice(c_off * HW, (c_off + cch) * HW)
                csl = slice(c_off, c_off + cch)
                c_off += cch
                for bt in range(2):
                    nc.sync.dma_start(
                        ft_tiles[bt][:, fsl],
                        featN[bt * 128:(bt + 1) * 128, fsl])
                for bt in range(2):
                    nc.vector.reduce_max(
                        cm_tiles[bt][:, csl],
                        ft_tiles[bt][:, fsl].rearrange(
                            "p (c r) -> p c r", r=HW),
                        axis=mybir.AxisListType.X,
                    )
                    # PCAM multiply on GpSimd: DVE owns reduce_max, POOL is
                    # otherwise idle in this phase.
                    nc.gpsimd.tensor_mul(
                        pcam_tiles[bt][:, fsl].rearrange(
                            "p (c r) -> p c r", r=HW),
                        ft_tiles[bt][:, fsl].rearrange(
                            "p (c r) -> p c r", r=HW),
                        cm_tiles[bt][:, csl].rearrange(
                            "p (c one) -> p c one", one=1)
                            .broadcast_to((128, cch, HW)),
                    )

            # ---- stage B: PE transposes + matmul1 ----
            # h accumulators in natural [b, d] orientation: per b-tile one
            # [128, 512] bank + one [128, 256] bank.
            hps = []
            for bt in range(2):
                hps.append((psH.tile([128, 512], F32, tag="hA",
                                     name=f"hA{bt}"),
                            psH.tile([128, 256], F32, tag="hB",
                                     name=f"hB{bt}")))

            def mm1(lhsT, rhs_w, start, stop):
                # out[b-half] += lhsT.T @ w1 rows; 512 + 256 split per bank
                for bt in range(2):
                    nc.tensor.matmul(
                        hps[bt][0][:], lhsT[:, bt * 128:(bt + 1) * 128],
                        rhs_w[:, 0:512], start=start, stop=stop)
                    nc.tensor.matmul(
                        hps[bt][1][:], lhsT[:, bt * 128:(bt + 1) * 128],
                        rhs_w[:, 512:768], start=start, stop=stop)

            # bias fold-in: ones-row (K=1) x b1row (core 0 carries b1,
            # other cores carry zeros so the ReduceScatter sums to one b1).
            ones_sb = constp.tile([1, 256], BF16, tag="ones")
            nc.gpsimd.memset(ones_sb[:], 1.0)
            b1_sb = constp.tile([1, D_HID], BF16, tag="b1r")
            nc.sync.dma_start(b1_sb[:], b1row[:])
            mm1(ones_sb[:], b1_sb[:], start=True, stop=False)

            at_tiles = {}

            def emit_transpose(k):
                pt = psB.tile([128, 256], BF16, tag="pt")
                at = atp.tile([128, 256], BF16, tag="at")
                for bt in range(2):
                    nc.tensor.transpose(
                        pt[:, bt * 128:(bt + 1) * 128],
                        pcam_tiles[bt][:, k * 128:(k + 1) * 128],
                        id_sb[:],
                    )
                nc.scalar.copy(at[:], pt[:])
                at_tiles[k] = at

            emit_transpose(0)
            emit_transpose(1)
            for k in range(K_TILES):
                if k + 2 < K_TILES:
                    emit_transpose(k + 2)
                w1_t = w1p.tile([128, D_HID], BF16, tag="w1")
                nc.sync.dma_start(w1_t[:], w1t[k * 128:(k + 1) * 128, :])
                at = at_tiles.pop(k)
                mm1(at[:], w1_t[:], start=False, stop=(k == K_TILES - 1))

            # ---- late constants for the epilogue ----
            w2_sb = constp.tile([128, D_TILES * N_CLS], BF16, tag="w2")
            nc.sync.dma_start(
                w2_sb[:].rearrange("p (t n) -> p t n", t=D_TILES),
                w2t[:].rearrange("(t p) n -> p t n", p=128))
            b2_sb = constp.tile([B_SH, N_CLS], F32, tag="b2")
            nc.sync.dma_start(b2_sb[:], b2b[:])

            # ---- stage C: ReduceScatter over the batch dim (bf16) ----
            inb = dramp.tile([B, D_HID], BF16, tag="inb")
            rsb = dramp.tile([B_SH, D_HID], BF16, tag="rsb")
            for bt in range(2):
                h_sb = hsbp.tile([128, D_HID], BF16, tag="hsb",
                                 name=f"hsb{bt}")
                nc.scalar.copy(h_sb[:, 0:512], hps[bt][0][:])
                nc.scalar.copy(h_sb[:, 512:768], hps[bt][1][:])
                nc.sync.dma_start(inb[bt * 128:(bt + 1) * 128, :], h_sb[:])
            nc.gpsimd.collective_compute(
                "ReduceScatter",
                mybir.AluOpType.add,
                replica_groups=[list(range(N_CORES))],
                ins=[inb[:].opt()],
                outs=[rsb[:].opt()],
            )

            # ---- stage D: relu on the local 32-row batch shard ----
            hsh = hsbp.tile([B_SH, D_HID], BF16, tag="hsh")
            nc.sync.dma_start(hsh[:], rsb[:])
            hrl = hsbp.tile([B_SH, D_HID], BF16, tag="hrl")
            nc.scalar.activation(
                hrl[:], hsh[:], mybir.ActivationFunctionType.Relu)

            # ---- stage E: logits shard = hrl @ W2.T + b2 ----
            # Need d on partitions: 6 small PE transposes of [32, 128].
            hT2 = hsbp.tile([128, D_TILES * B_SH], BF16, tag="hT2")
            for t in range(D_TILES):
                ptt = psB.tile([128, B_SH], BF16, tag="pt", name=f"ptt{t}")
                nc.tensor.transpose(
                    ptt[:], hrl[:, t * 128:(t + 1) * 128],
                    id_sb[0:B_SH, 0:B_SH])
                nc.scalar.copy(hT2[:, t * B_SH:(t + 1) * B_SH], ptt[:])
            lg = psB.tile([B_SH, N_CLS], F32, tag="pt", name="lgps")
            for t in range(D_TILES):
                nc.tensor.matmul(
                    lg[:],
                    hT2[:, t * B_SH:(t + 1) * B_SH],
                    w2_sb[:, t * N_CLS:(t + 1) * N_CLS],
                    start=(t == 0),
                    stop=(t == D_TILES - 1),
                )
            out_sb = hsbp.tile([B_SH, N_CLS], F32, tag="outsb")
            nc.vector.tensor_add(out_sb[:], lg[:], b2_sb[:])
            nc.sync.dma_start(logits_sh[:], out_sb[:])

    nc.compile()
    return nc


def _prep_inputs(features, W1, b1, W2, b2):
    bf16 = ml_dtypes.bfloat16
    feat_flat = np.asarray(features, np.float32).reshape(B, C, HW)
    W1 = np.asarray(W1, np.float32)
    b1 = np.asarray(b1, np.float32)
    w2t_full = np.ascontiguousarray(
        np.asarray(W2, np.float32).T.astype(bf16))       # [768, 5]
    b2b = np.ascontiguousarray(np.broadcast_to(
        np.asarray(b2, np.float32), (B_SH, N_CLS)))
    ident = np.eye(128, dtype=bf16)
    b1_bf = b1.astype(bf16).reshape(1, D_HID)
    zeros_row = np.zeros((1, D_HID), bf16)

    in_maps = []
    for c in range(N_CORES):
        featN = np.ascontiguousarray(
            feat_flat[:, c * C_PER:(c + 1) * C_PER, :]).reshape(B, F_PER)
        w1t = np.ascontiguousarray(
            W1[:, c * F_PER:(c + 1) * F_PER].T)
        in_maps.append({
            "featN": featN.astype(bf16),
            "w1t": w1t.astype(bf16),
            "b1row": b1_bf if c == 0 else zeros_row,
            "w2t": w2t_full,
            "b2b": b2b,
            "ident": ident,
        })
    return in_maps


def kernel(features, W1, b1, W2, b2):
    if "nc" not in _CACHE:
        _CACHE["nc"] = _build_program()
    nc = _CACHE["nc"]
    in_maps = _prep_inputs(features, W1, b1, W2, b2)
    res = run_bass_kernel_spmd(nc, in_maps, core_ids=list(range(N_CORES)))
    kernel._last_results = res
    logits = np.concatenate(
        [np.asarray(res.results[c]["logits_sh"], np.float32)
         for c in range(N_CORES)], axis=0)
    return (logits, np.asarray(features, np.float32))


# revision 63
# speedup vs baseline: 1.0741x; 1.0741x over previous
"""Bass/Trainium2 kernel for DenseNetWithPCAM (pooling head + 2-layer MLP).

Computation (reference):
    ch_max = max(features, axis=(2,3))            # [B, C]
    pcams  = features * ch_max[..., None, None]   # [B, C, H, W]
    h      = relu(pcams.reshape(B, -1) @ W1.T + b1)
    logits = h @ W2.T + b2
    return (logits, features)

Sharding (8 NeuronCores, tensor-parallel over the contraction dim):
    Core c owns channels [128c, 128(c+1)) -> flat cols [6272c, 6272(c+1)).
    Per core: PCAM on-chip (reduce_max + broadcast multiply, bf16),
    PE-transpose pcams into [f, b] tiles, then matmul with the W1 shard
    streamed as the moving operand -> partial h [256 b, 768 d] in natural
    orientation (b1 folded in as a ones-row on core 0 only).  One bf16
    ReduceScatter over the batch dim gives each core 32 fully-reduced rows;
    relu + W2 matmul produce a [32, 5] logits shard per core and the host
    concatenates the 8 shards.  Features pass through on the host.
"""

import sys
import types

import numpy as np
import ml_dtypes

import concourse.bass as bass
import concourse.mybir as mybir
import concourse.tile as tile
from concourse import bacc
from concourse.bass_utils import run_bass_kernel_spmd

# bass_utils' BASS_TRACE path imports antenv.axon_hooks, which this
# container's antenv stub does not ship; registering a no-op hook keeps a
# trace-enabled environment from crashing (it just skips NTFF capture).
try:
    import antenv.axon_hooks  # noqa: F401
except ImportError:
    _m = types.ModuleType("antenv.axon_hooks")
    _m.get_axon_ntff_profile_hook = lambda: None
    sys.modules["antenv.axon_hooks"] = _m

# Problem shape (hardcoded per contract)
B, C, H, W_SP = 256, 1024, 7, 7
HW = H * W_SP                    # 49
D_HID, N_CLS = 768, 5
N_CORES = 8
C_PER = C // N_CORES             # 128 channels per core
F_PER = C_PER * HW               # 6272 flat features per core
K_TILES = F_PER // 128           # 49 contraction tiles of 128
D_TILES = D_HID // 128           # 6
B_SH = B // N_CORES              # 32 batch rows per core after RS
BF16 = mybir.dt.bfloat16
F32 = mybir.dt.float32

_CACHE = {}


def _build_program():
    nc = bacc.Bacc("TRN2", target_bir_lowering=False, debug=False,
                   num_devices=N_CORES)

    featN = nc.dram_tensor("featN", [B, F_PER], BF16, kind="ExternalInput")
    w1t = nc.dram_tensor("w1t", [F_PER, D_HID], BF16, kind="ExternalInput")
    b1row = nc.dram_tensor("b1row", [1, D_HID], BF16, kind="ExternalInput")
    w2t = nc.dram_tensor("w2t", [D_HID, N_CLS], BF16, kind="ExternalInput")
    b2b = nc.dram_tensor("b2b", [B_SH, N_CLS], F32, kind="ExternalInput")
    ident = nc.dram_tensor("ident", [128, 128], BF16, kind="ExternalInput")
    logits_sh = nc.dram_tensor("logits_sh", [B_SH, N_CLS], F32,
                               kind="ExternalOutput")

    with tile.TileContext(nc) as tc:
        with (
            tc.tile_pool(name="const", bufs=1) as constp,
            tc.tile_pool(name="feat", bufs=2) as featp,
            tc.tile_pool(name="cmax", bufs=2) as cmaxp,
            tc.tile_pool(name="pcam", bufs=2) as pcamp,
            tc.tile_pool(name="w1", bufs=12) as w1p,
            tc.tile_pool(name="at", bufs=8) as atp,
            tc.tile_pool(name="hsb", bufs=1) as hsbp,
            tc.tile_pool(name="psH", bufs=2, space="PSUM") as psH,
            tc.tile_pool(name="psB", bufs=4, space="PSUM") as psB,
            tc.tile_pool(name="dram", bufs=1, space="DRAM") as dramp,
        ):
            # ---- constants (ident on SWDGE so the sync FIFO leads with
            # the first feature chunk) ----
            id_sb = constp.tile([128, 128], BF16, tag="ident")
            nc.gpsimd.dma_start(id_sb[:], ident[:])

            # ---- stage A: load features, per-channel max, pcam multiply ----
            # Chunked over channels so the DVE/GpSimd work pipelines under
            # the PE stage (Tile tracks subtile deps).
            CHUNKS = [12, 20, 48, 48]
            ft_tiles, cm_tiles, pcam_tiles = [], [], []
            for bt in range(2):
                ft_tiles.append(featp.tile([128, F_PER], BF16, tag="feat",
                                           name=f"ft{bt}"))
                cm_tiles.append(cmaxp.tile([128, C_PER], BF16, tag="cmax",
                                           name=f"cm{bt}"))
                pcam_tiles.append(pcamp.tile([128, F_PER], BF16, tag="pcam",
                                             name=f"pc{bt}"))
            import contextlib
            c_off = 0
            for ci, cch in enumerate(CHUNKS):
                fsl = slice(c_off * HW, (c_off + cch) * HW)
                csl = slice(c_off, c_off + cch)
                c_off += cch
                # chunk 0 feeds the very first PE transposes: pin it to the
                # front of the scheduler's priority order so later chunks'
                # reduces don't preempt its multiply on the DVE.
                prio = tc.high_priority() if ci == 0 else contextlib.nullcontext()
                with prio:
                    for bt in range(2):
                        nc.sync.dma_start(
                            ft_tiles[bt][:, fsl],
                            featN[bt * 128:(bt + 1) * 128, fsl])
                    for bt in range(2):
                        nc.vector.reduce_max(
                            cm_tiles[bt][:, csl],
                            ft_tiles[bt][:, fsl].rearrange(
                                "p (c r) -> p c r", r=HW),
                            axis=mybir.AxisListType.X,
                        )
                        # PCAM multiply on GpSimd: DVE owns reduce_max, POOL
                        # is otherwise idle.  First chunk's bt=1 goes to DVE
                        # so the two halves finish in parallel.
                        eng = nc.vector if (ci == 0 and bt == 1) \
                            else nc.gpsimd
                        eng.tensor_mul(
                            pcam_tiles[bt][:, fsl].rearrange(
                                "p (c r) -> p c r", r=HW),
                            ft_tiles[bt][:, fsl].rearrange(
                                "p (c r) -> p c r", r=HW),
                            cm_tiles[bt][:, csl].rearrange(
                                "p (c one) -> p c one", one=1)
                                .broadcast_to((128, cch, HW)),
                        )

            # ---- stage B: PE transposes + matmul1 ----
            # h accumulators in natural [b, d] orientation: per b-tile one
            # [128, 512] bank + one [128, 256] bank.
            hps = []
            for bt in range(2):
                hps.append((psH.tile([128, 512], F32, tag="hA",
                                     name=f"hA{bt}"),
                            psH.tile([128, 256], F32, tag="hB",
                                     name=f"hB{bt}")))

            def mm1(lhsT, rhs_w, start, stop):
                # out[b-half] += lhsT.T @ w1 rows; 512 + 256 split per bank
                for bt in range(2):
                    nc.tensor.matmul(
                        hps[bt][0][:], lhsT[:, bt * 128:(bt + 1) * 128],
                        rhs_w[:, 0:512], start=start, stop=stop)
                    nc.tensor.matmul(
                        hps[bt][1][:], lhsT[:, bt * 128:(bt + 1) * 128],
                        rhs_w[:, 512:768], start=start, stop=stop)

            # bias fold-in: ones-row (K=1) x b1row (core 0 carries b1,
            # other cores carry zeros so the ReduceScatter sums to one b1).
            ones_sb = constp.tile([1, 256], BF16, tag="ones")
            nc.gpsimd.memset(ones_sb[:], 1.0)
            b1_sb = constp.tile([1, D_HID], BF16, tag="b1r")
            nc.gpsimd.dma_start(b1_sb[:], b1row[:])
            mm1(ones_sb[:], b1_sb[:], start=True, stop=False)

            at_tiles = {}

            def emit_transpose(k):
                pt = psB.tile([128, 256], BF16, tag="pt")
                at = atp.tile([128, 256], BF16, tag="at")
                for bt in range(2):
                    nc.tensor.transpose(
                        pt[:, bt * 128:(bt + 1) * 128],
                        pcam_tiles[bt][:, k * 128:(k + 1) * 128],
                        id_sb[:],
                    )
                nc.scalar.copy(at[:], pt[:])
                at_tiles[k] = at

            emit_transpose(0)
            emit_transpose(1)
            for k in range(K_TILES):
                if k + 2 < K_TILES:
                    emit_transpose(k + 2)
                w1_t = w1p.tile([128, D_HID], BF16, tag="w1")
                nc.sync.dma_start(w1_t[:], w1t[k * 128:(k + 1) * 128, :])
                at = at_tiles.pop(k)
                mm1(at[:], w1_t[:], start=False, stop=(k == K_TILES - 1))

            # ---- late constants for the epilogue ----
            w2_sb = constp.tile([128, D_TILES * N_CLS], BF16, tag="w2")
            nc.sync.dma_start(
                w2_sb[:].rearrange("p (t n) -> p t n", t=D_TILES),
                w2t[:].rearrange("(t p) n -> p t n", p=128))
            b2_sb = constp.tile([B_SH, N_CLS], F32, tag="b2")
            nc.sync.dma_start(b2_sb[:], b2b[:])

            # ---- stage C: ReduceScatter over the batch dim (bf16) ----
            inb = dramp.tile([B, D_HID], BF16, tag="inb")
            rsb = dramp.tile([B_SH, D_HID], BF16, tag="rsb")
            for bt in range(2):
                h_sb = hsbp.tile([128, D_HID], BF16, tag="hsb",
                                 name=f"hsb{bt}", bufs=2)
                # evacuate the two banks on different engines, in parallel
                nc.scalar.copy(h_sb[:, 0:512], hps[bt][0][:])
                nc.vector.tensor_copy(h_sb[:, 512:768], hps[bt][1][:])
                nc.sync.dma_start(inb[bt * 128:(bt + 1) * 128, :], h_sb[:])
            nc.gpsimd.collective_compute(
                "ReduceScatter",
                mybir.AluOpType.add,
                replica_groups=[list(range(N_CORES))],
                ins=[inb[:].opt()],
                outs=[rsb[:].opt()],
            )

            # ---- stage D: relu on the local 32-row batch shard ----
            hsh = hsbp.tile([B_SH, D_HID], BF16, tag="hsh")
            nc.sync.dma_start(hsh[:], rsb[:])
            hrl = hsbp.tile([B_SH, D_HID], BF16, tag="hrl")
            nc.scalar.activation(
                hrl[:], hsh[:], mybir.ActivationFunctionType.Relu)

            # ---- stage E: logits shard = hrl @ W2.T + b2 ----
            # Need d on partitions: 6 small PE transposes of [32, 128].
            hT2 = hsbp.tile([128, D_TILES * B_SH], BF16, tag="hT2")
            for t in range(D_TILES):
                ptt = psB.tile([128, B_SH], BF16, tag="pt", name=f"ptt{t}")
                nc.tensor.transpose(
                    ptt[:], hrl[:, t * 128:(t + 1) * 128],
                    id_sb[0:B_SH, 0:B_SH])
                nc.scalar.copy(hT2[:, t * B_SH:(t + 1) * B_SH], ptt[:])
            lg = psB.tile([B_SH, N_CLS], F32, tag="pt", name="lgps")
            for t in range(D_TILES):
                nc.tensor.matmul(
                    lg[:],
                    hT2[:, t * B_SH:(t + 1) * B_SH],
                    w2_sb[:, t * N_CLS:(t + 1) * N_CLS],
                    start=(t == 0),
                    stop=(t == D_TILES - 1),
                )
            out_sb = hsbp.tile([B_SH, N_CLS], F32, tag="outsb")
            nc.vector.tensor_add(out_sb[:], lg[:], b2_sb[:])
            nc.sync.dma_start(logits_sh[:], out_sb[:])

    nc.compile()
    return nc


def _prep_inputs(features, W1, b1, W2, b2):
    bf16 = ml_dtypes.bfloat16
    feat_flat = np.asarray(features, np.float32).reshape(B, C, HW)
    W1 = np.asarray(W1, np.float32)
    b1 = np.asarray(b1, np.float32)
    w2t_full = np.ascontiguousarray(
        np.asarray(W2, np.float32).T.astype(bf16))       # [768, 5]
    b2b = np.ascontiguousarray(np.broadcast_to(
        np.asarray(b2, np.float32), (B_SH, N_CLS)))
    ident = np.eye(128, dtype=bf16)
    b1_bf = b1.astype(bf16).reshape(1, D_HID)
    zeros_row = np.zeros((1, D_HID), bf16)

    in_maps = []
    for c in range(N_CORES):
        featN = np.ascontiguousarray(
            feat_flat[:, c * C_PER:(c + 1) * C_PER, :]).reshape(B, F_PER)
        w1t = np.ascontiguousarray(
            W1[:, c * F_PER:(c + 1) * F_PER].T)
        in_maps.append({
            "featN": featN.astype(bf16),
            "w1t": w1t.astype(bf16),
            "b1row": b1_bf if c == 0 else zeros_row,
            "w2t": w2t_full,
            "b2b": b2b,
            "ident": ident,
        })
    return in_maps


def kernel(features, W1, b1, W2, b2):
    if "nc" not in _CACHE:
        _CACHE["nc"] = _build_program()
    nc = _CACHE["nc"]
    in_maps = _prep_inputs(features, W1, b1, W2, b2)
    res = run_bass_kernel_spmd(nc, in_maps, core_ids=list(range(N_CORES)))
    kernel._last_results = res
    logits = np.concatenate(
        [np.asarray(res.results[c]["logits_sh"], np.float32)
         for c in range(N_CORES)], axis=0)
    return (logits, np.asarray(features, np.float32))


# revision 64
# speedup vs baseline: 1.0886x; 1.0135x over previous
"""Bass/Trainium2 kernel for DenseNetWithPCAM (pooling head + 2-layer MLP).

Computation (reference):
    ch_max = max(features, axis=(2,3))            # [B, C]
    pcams  = features * ch_max[..., None, None]   # [B, C, H, W]
    h      = relu(pcams.reshape(B, -1) @ W1.T + b1)
    logits = h @ W2.T + b2
    return (logits, features)

Sharding (8 NeuronCores, tensor-parallel over the contraction dim):
    Core c owns channels [128c, 128(c+1)) -> flat cols [6272c, 6272(c+1)).
    Per core: PCAM on-chip (reduce_max + broadcast multiply, bf16),
    PE-transpose pcams into [f, b] tiles, then matmul with the W1 shard
    streamed as the moving operand -> partial h [256 b, 768 d] in natural
    orientation (b1 folded in as a ones-row on core 0 only).  One bf16
    ReduceScatter over the batch dim gives each core 32 fully-reduced rows;
    relu + W2 matmul produce a [32, 5] logits shard per core and the host
    concatenates the 8 shards.  Features pass through on the host.
"""

import sys
import types

import numpy as np
import ml_dtypes

import concourse.bass as bass
import concourse.mybir as mybir
import concourse.tile as tile
from concourse import bacc
from concourse.bass_utils import run_bass_kernel_spmd

# bass_utils' BASS_TRACE path imports antenv.axon_hooks, which this
# container's antenv stub does not ship; registering a no-op hook keeps a
# trace-enabled environment from crashing (it just skips NTFF capture).
try:
    import antenv.axon_hooks  # noqa: F401
except ImportError:
    _m = types.ModuleType("antenv.axon_hooks")
    _m.get_axon_ntff_profile_hook = lambda: None
    sys.modules["antenv.axon_hooks"] = _m

# Problem shape (hardcoded per contract)
B, C, H, W_SP = 256, 1024, 7, 7
HW = H * W_SP                    # 49
D_HID, N_CLS = 768, 5
N_CORES = 8
C_PER = C // N_CORES             # 128 channels per core
F_PER = C_PER * HW               # 6272 flat features per core
K_TILES = F_PER // 128           # 49 contraction tiles of 128
D_TILES = D_HID // 128           # 6
B_SH = B // N_CORES              # 32 batch rows per core after RS
BF16 = mybir.dt.bfloat16
F32 = mybir.dt.float32

_CACHE = {}


def _build_program():
    nc = bacc.Bacc("TRN2", target_bir_lowering=False, debug=False,
                   num_devices=N_CORES)

    featN = nc.dram_tensor("featN", [B, F_PER], BF16, kind="ExternalInput")
    w1t = nc.dram_tensor("w1t", [F_PER, D_HID], BF16, kind="ExternalInput")
    b1row = nc.dram_tensor("b1row", [1, D_HID], BF16, kind="ExternalInput")
    w2t = nc.dram_tensor("w2t", [D_HID, N_CLS], BF16, kind="ExternalInput")
    b2b = nc.dram_tensor("b2b", [B_SH, N_CLS], F32, kind="ExternalInput")
    ident = nc.dram_tensor("ident", [128, 128], BF16, kind="ExternalInput")
    logits_sh = nc.dram_tensor("logits_sh", [B_SH, N_CLS], F32,
                               kind="ExternalOutput")

    with tile.TileContext(nc) as tc:
        with (
            tc.tile_pool(name="const", bufs=1) as constp,
            tc.tile_pool(name="feat", bufs=2) as featp,
            tc.tile_pool(name="cmax", bufs=2) as cmaxp,
            tc.tile_pool(name="pcam", bufs=2) as pcamp,
            tc.tile_pool(name="w1", bufs=12) as w1p,
            tc.tile_pool(name="at", bufs=8) as atp,
            tc.tile_pool(name="hsb", bufs=1) as hsbp,
            tc.tile_pool(name="psH", bufs=2, space="PSUM") as psH,
            tc.tile_pool(name="psB", bufs=4, space="PSUM") as psB,
            tc.tile_pool(name="dram", bufs=1, space="DRAM") as dramp,
        ):
            # ---- constants (ident on SWDGE so the sync FIFO leads with
            # the first feature chunk) ----
            id_sb = constp.tile([128, 128], BF16, tag="ident")
            nc.gpsimd.dma_start(id_sb[:], ident[:])

            # ---- stage A: load features, per-channel max, pcam multiply ----
            # Chunked over channels so the DVE/GpSimd work pipelines under
            # the PE stage (Tile tracks subtile deps).
            CHUNKS = [12, 20, 48, 48]
            ft_tiles, cm_tiles, pcam_tiles = [], [], []
            for bt in range(2):
                ft_tiles.append(featp.tile([128, F_PER], BF16, tag="feat",
                                           name=f"ft{bt}"))
                cm_tiles.append(cmaxp.tile([128, C_PER], BF16, tag="cmax",
                                           name=f"cm{bt}"))
                pcam_tiles.append(pcamp.tile([128, F_PER], BF16, tag="pcam",
                                             name=f"pc{bt}"))
            import contextlib
            c_off = 0
            for ci, cch in enumerate(CHUNKS):
                fsl = slice(c_off * HW, (c_off + cch) * HW)
                csl = slice(c_off, c_off + cch)
                c_off += cch
                # chunk 0 feeds the very first PE transposes: pin it to the
                # front of the scheduler's priority order so later chunks'
                # reduces don't preempt its multiply on the DVE.
                prio = tc.high_priority() if ci == 0 else contextlib.nullcontext()
                with prio:
                    for bt in range(2):
                        nc.sync.dma_start(
                            ft_tiles[bt][:, fsl],
                            featN[bt * 128:(bt + 1) * 128, fsl])
                    for bt in range(2):
                        nc.vector.reduce_max(
                            cm_tiles[bt][:, csl],
                            ft_tiles[bt][:, fsl].rearrange(
                                "p (c r) -> p c r", r=HW),
                            axis=mybir.AxisListType.X,
                        )
                        # PCAM multiply on GpSimd: DVE owns reduce_max, POOL
                        # is otherwise idle.  First chunk's bt=1 goes to DVE
                        # so the two halves finish in parallel.
                        eng = nc.vector if (ci == 0 and bt == 1) \
                            else nc.gpsimd
                        eng.tensor_mul(
                            pcam_tiles[bt][:, fsl].rearrange(
                                "p (c r) -> p c r", r=HW),
                            ft_tiles[bt][:, fsl].rearrange(
                                "p (c r) -> p c r", r=HW),
                            cm_tiles[bt][:, csl].rearrange(
                                "p (c one) -> p c one", one=1)
                                .broadcast_to((128, cch, HW)),
                        )

            # ---- stage B: PE transposes + matmul1 ----
            # h accumulators in natural [b, d] orientation: per b-tile one
            # [128, 512] bank + one [128, 256] bank.
            hps = []
            for bt in range(2):
                hps.append((psH.tile([128, 512], F32, tag="hA",
                                     name=f"hA{bt}"),
                            psH.tile([128, 256], F32, tag="hB",
                                     name=f"hB{bt}")))

            def mm1(lhsT, rhs_w, start, stop):
                # out[b-half] += lhsT.T @ w1 rows; 512 + 256 split per bank
                for bt in range(2):
                    nc.tensor.matmul(
                        hps[bt][0][:], lhsT[:, bt * 128:(bt + 1) * 128],
                        rhs_w[:, 0:512], start=start, stop=stop)
                    nc.tensor.matmul(
                        hps[bt][1][:], lhsT[:, bt * 128:(bt + 1) * 128],
                        rhs_w[:, 512:768], start=start, stop=stop)

            # bias fold-in: ones-row (K=1) x b1row (core 0 carries b1,
            # other cores carry zeros so the ReduceScatter sums to one b1).
            ones_sb = constp.tile([1, 256], BF16, tag="ones")
            nc.gpsimd.memset(ones_sb[:], 1.0)
            b1_sb = constp.tile([1, D_HID], BF16, tag="b1r")
            nc.gpsimd.dma_start(b1_sb[:], b1row[:])
            mm1(ones_sb[:], b1_sb[:], start=True, stop=False)

            at_tiles = {}

            def emit_transpose(k):
                pt = psB.tile([128, 256], BF16, tag="pt")
                at = atp.tile([128, 256], BF16, tag="at")
                for bt in range(2):
                    nc.tensor.transpose(
                        pt[:, bt * 128:(bt + 1) * 128],
                        pcam_tiles[bt][:, k * 128:(k + 1) * 128],
                        id_sb[:],
                    )
                nc.scalar.copy(at[:], pt[:])
                at_tiles[k] = at

            emit_transpose(0)
            emit_transpose(1)
            for k in range(K_TILES):
                if k + 2 < K_TILES:
                    emit_transpose(k + 2)
                w1_t = w1p.tile([128, D_HID], BF16, tag="w1")
                nc.sync.dma_start(w1_t[:], w1t[k * 128:(k + 1) * 128, :])
                at = at_tiles.pop(k)
                mm1(at[:], w1_t[:], start=False, stop=(k == K_TILES - 1))

            # ---- late constants for the epilogue ----
            w2_sb = constp.tile([128, D_TILES * N_CLS], BF16, tag="w2")
            nc.sync.dma_start(
                w2_sb[:].rearrange("p (t n) -> p t n", t=D_TILES),
                w2t[:].rearrange("(t p) n -> p t n", p=128))
            b2_sb = constp.tile([B_SH, N_CLS], F32, tag="b2")
            nc.sync.dma_start(b2_sb[:], b2b[:])

            # ---- stage C: ReduceScatter over the batch dim (bf16) ----
            inb = dramp.tile([B, D_HID], BF16, tag="inb")
            rsb = dramp.tile([B_SH, D_HID], BF16, tag="rsb")
            for bt in range(2):
                h_sb = hsbp.tile([128, D_HID], BF16, tag="hsb",
                                 name=f"hsb{bt}", bufs=2)
                # evacuate the two banks on different engines, in parallel
                nc.scalar.copy(h_sb[:, 0:512], hps[bt][0][:])
                nc.vector.tensor_copy(h_sb[:, 512:768], hps[bt][1][:])
                nc.sync.dma_start(inb[bt * 128:(bt + 1) * 128, :], h_sb[:])
            nc.gpsimd.collective_compute(
                "ReduceScatter",
                mybir.AluOpType.add,
                replica_groups=[list(range(N_CORES))],
                ins=[inb[:].opt()],
                outs=[rsb[:].opt()],
            )

            # ---- stages D+E: transpose the RS shard to d-major, fusing
            # the relu into the PSUM evacuation (relu commutes with
            # transpose; b1 is already folded into h).
            hsh = hsbp.tile([B_SH, D_HID], BF16, tag="hsh")
            nc.sync.dma_start(hsh[:], rsb[:])
            hT2 = hsbp.tile([128, D_TILES * B_SH], BF16, tag="hT2")
            for t in range(D_TILES):
                ptt = psB.tile([128, B_SH], BF16, tag="pt", name=f"ptt{t}")
                nc.tensor.transpose(
                    ptt[:], hsh[:, t * 128:(t + 1) * 128],
                    id_sb[0:B_SH, 0:B_SH])
                nc.scalar.activation(
                    hT2[:, t * B_SH:(t + 1) * B_SH], ptt[:],
                    mybir.ActivationFunctionType.Relu)
            lg = psB.tile([B_SH, N_CLS], F32, tag="pt", name="lgps")
            for t in range(D_TILES):
                nc.tensor.matmul(
                    lg[:],
                    hT2[:, t * B_SH:(t + 1) * B_SH],
                    w2_sb[:, t * N_CLS:(t + 1) * N_CLS],
                    start=(t == 0),
                    stop=(t == D_TILES - 1),
                )
            out_sb = hsbp.tile([B_SH, N_CLS], F32, tag="outsb")
            nc.vector.tensor_add(out_sb[:], lg[:], b2_sb[:])
            nc.sync.dma_start(logits_sh[:], out_sb[:])

    nc.compile()
    return nc


def _prep_inputs(features, W1, b1, W2, b2):
    bf16 = ml_dtypes.bfloat16
    feat_flat = np.asarray(features, np.float32).reshape(B, C, HW)
    W1 = np.asarray(W1, np.float32)
    b1 = np.asarray(b1, np.float32)
    w2t_full = np.ascontiguousarray(
        np.asarray(W2, np.float32).T.astype(bf16))       # [768, 5]
    b2b = np.ascontiguousarray(np.broadcast_to(
        np.asarray(b2, np.float32), (B_SH, N_CLS)))
    ident = np.eye(128, dtype=bf16)
    b1_bf = b1.astype(bf16).reshape(1, D_HID)
    zeros_row = np.zeros((1, D_HID), bf16)

    in_maps = []
    for c in range(N_CORES):
        featN = np.ascontiguousarray(
            feat_flat[:, c * C_PER:(c + 1) * C_PER, :]).reshape(B, F_PER)
        w1t = np.ascontiguousarray(
            W1[:, c * F_PER:(c + 1) * F_PER].T)
        in_maps.append({
            "featN": featN.astype(bf16),
            "w1t": w1t.astype(bf16),
            "b1row": b1_bf if c == 0 else zeros_row,
            "w2t": w2t_full,
            "b2b": b2b,
            "ident": ident,
        })
    return in_maps


def kernel(features, W1, b1, W2, b2):
    if "nc" not in _CACHE:
        _CACHE["nc"] = _build_program()
    nc = _CACHE["nc"]
    in_maps = _prep_inputs(features, W1, b1, W2, b2)
    res = run_bass_kernel_spmd(nc, in_maps, core_ids=list(range(N_CORES)))
    kernel._last_results = res
    logits = np.concatenate(
        [np.asarray(res.results[c]["logits_sh"], np.float32)
         for c in range(N_CORES)], axis=0)
    return (logits, np.asarray(features, np.float32))


# revision 69
# speedup vs baseline: 1.0956x; 1.0064x over previous
"""Bass/Trainium2 kernel for DenseNetWithPCAM (pooling head + 2-layer MLP).

Computation (reference):
    ch_max = max(features, axis=(2,3))            # [B, C]
    pcams  = features * ch_max[..., None, None]   # [B, C, H, W]
    h      = relu(pcams.reshape(B, -1) @ W1.T + b1)
    logits = h @ W2.T + b2
    return (logits, features)

Sharding (8 NeuronCores, tensor-parallel over the contraction dim):
    Core c owns channels [128c, 128(c+1)) -> flat cols [6272c, 6272(c+1)).
    Per core: PCAM on-chip (reduce_max + broadcast multiply, bf16),
    PE-transpose pcams into [f, b] tiles, then matmul with the W1 shard
    streamed as the moving operand -> partial h [256 b, 768 d] in natural
    orientation (b1 folded in as a ones-row on core 0 only).  One bf16
    ReduceScatter over the batch dim gives each core 32 fully-reduced rows;
    relu + W2 matmul produce a [32, 5] logits shard per core and the host
    concatenates the 8 shards.  Features pass through on the host.
"""

import sys
import types

import numpy as np
import ml_dtypes

import concourse.bass as bass
import concourse.mybir as mybir
import concourse.tile as tile
from concourse import bacc
from concourse.bass_utils import run_bass_kernel_spmd

# bass_utils' BASS_TRACE path imports antenv.axon_hooks, which this
# container's antenv stub does not ship; registering a no-op hook keeps a
# trace-enabled environment from crashing (it just skips NTFF capture).
try:
    import antenv.axon_hooks  # noqa: F401
except ImportError:
    _m = types.ModuleType("antenv.axon_hooks")
    _m.get_axon_ntff_profile_hook = lambda: None
    sys.modules["antenv.axon_hooks"] = _m

# Problem shape (hardcoded per contract)
B, C, H, W_SP = 256, 1024, 7, 7
HW = H * W_SP                    # 49
D_HID, N_CLS = 768, 5
N_CORES = 8
C_PER = C // N_CORES             # 128 channels per core
F_PER = C_PER * HW               # 6272 flat features per core
K_TILES = F_PER // 128           # 49 contraction tiles of 128
D_TILES = D_HID // 128           # 6
B_SH = B // N_CORES              # 32 batch rows per core after RS
BF16 = mybir.dt.bfloat16
F32 = mybir.dt.float32

_CACHE = {}


def _build_program():
    nc = bacc.Bacc("TRN2", target_bir_lowering=False, debug=False,
                   num_devices=N_CORES)

    featN = nc.dram_tensor("featN", [B, F_PER], BF16, kind="ExternalInput")
    w1t = nc.dram_tensor("w1t", [F_PER, D_HID], BF16, kind="ExternalInput")
    b1row = nc.dram_tensor("b1row", [1, D_HID], BF16, kind="ExternalInput")
    w2t = nc.dram_tensor("w2t", [D_HID, N_CLS], BF16, kind="ExternalInput")
    b2b = nc.dram_tensor("b2b", [B_SH, N_CLS], F32, kind="ExternalInput")
    ident = nc.dram_tensor("ident", [128, 128], BF16, kind="ExternalInput")
    logits_sh = nc.dram_tensor("logits_sh", [B_SH, N_CLS], F32,
                               kind="ExternalOutput")

    with tile.TileContext(nc) as tc:
        with (
            tc.tile_pool(name="const", bufs=1) as constp,
            tc.tile_pool(name="feat", bufs=2) as featp,
            tc.tile_pool(name="cmax", bufs=2) as cmaxp,
            tc.tile_pool(name="pcam", bufs=2) as pcamp,
            tc.tile_pool(name="w1", bufs=12) as w1p,
            tc.tile_pool(name="at", bufs=8) as atp,
            tc.tile_pool(name="hsb", bufs=1) as hsbp,
            tc.tile_pool(name="psH", bufs=2, space="PSUM") as psH,
            tc.tile_pool(name="psB", bufs=4, space="PSUM") as psB,
            tc.tile_pool(name="dram", bufs=1, space="DRAM") as dramp,
        ):
            # ---- constants (ident on SWDGE so the sync FIFO leads with
            # the first feature chunk) ----
            id_sb = constp.tile([128, 128], BF16, tag="ident")
            nc.gpsimd.dma_start(id_sb[:], ident[:])

            # ---- stage A: load features, per-channel max, pcam multiply ----
            # Chunked over channels so the DVE/GpSimd work pipelines under
            # the PE stage (Tile tracks subtile deps).
            CHUNKS = [12, 20, 24, 24, 24, 24]
            ft_tiles, cm_tiles, pcam_tiles = [], [], []
            for bt in range(2):
                ft_tiles.append(featp.tile([128, F_PER], BF16, tag="feat",
                                           name=f"ft{bt}"))
                cm_tiles.append(cmaxp.tile([128, C_PER], BF16, tag="cmax",
                                           name=f"cm{bt}"))
                pcam_tiles.append(pcamp.tile([128, F_PER], BF16, tag="pcam",
                                             name=f"pc{bt}"))
            import contextlib
            c_off = 0
            for ci, cch in enumerate(CHUNKS):
                fsl = slice(c_off * HW, (c_off + cch) * HW)
                csl = slice(c_off, c_off + cch)
                c_off += cch
                # chunk 0 feeds the very first PE transposes: pin it to the
                # front of the scheduler's priority order so later chunks'
                # reduces don't preempt its multiply on the DVE.
                prio = tc.high_priority() if ci == 0 else contextlib.nullcontext()
                with prio:
                    for bt in range(2):
                        nc.sync.dma_start(
                            ft_tiles[bt][:, fsl],
                            featN[bt * 128:(bt + 1) * 128, fsl])
                    for bt in range(2):
                        nc.vector.reduce_max(
                            cm_tiles[bt][:, csl],
                            ft_tiles[bt][:, fsl].rearrange(
                                "p (c r) -> p c r", r=HW),
                            axis=mybir.AxisListType.X,
                        )
                        # PCAM multiply on GpSimd: DVE owns reduce_max, POOL
                        # is otherwise idle.  First chunk's bt=1 goes to DVE
                        # so the two halves finish in parallel.
                        eng = nc.vector if (ci == 0 and bt == 1) \
                            else nc.gpsimd
                        eng.tensor_mul(
                            pcam_tiles[bt][:, fsl].rearrange(
                                "p (c r) -> p c r", r=HW),
                            ft_tiles[bt][:, fsl].rearrange(
                                "p (c r) -> p c r", r=HW),
                            cm_tiles[bt][:, csl].rearrange(
                                "p (c one) -> p c one", one=1)
                                .broadcast_to((128, cch, HW)),
                        )

            # ---- stage B: PE transposes + matmul1 ----
            # h accumulators in natural [b, d] orientation: per b-tile one
            # [128, 512] bank + one [128, 256] bank.
            hps = []
            for bt in range(2):
                hps.append((psH.tile([128, 512], F32, tag="hA",
                                     name=f"hA{bt}"),
                            psH.tile([128, 256], F32, tag="hB",
                                     name=f"hB{bt}")))

            def mm1(lhsT, rhs_w, start, stop):
                # out[b-half] += lhsT.T @ w1 rows; 512 + 256 split per bank
                for bt in range(2):
                    nc.tensor.matmul(
                        hps[bt][0][:], lhsT[:, bt * 128:(bt + 1) * 128],
                        rhs_w[:, 0:512], start=start, stop=stop)
                    nc.tensor.matmul(
                        hps[bt][1][:], lhsT[:, bt * 128:(bt + 1) * 128],
                        rhs_w[:, 512:768], start=start, stop=stop)

            # bias fold-in: ones-row (K=1) x b1row (core 0 carries b1,
            # other cores carry zeros so the ReduceScatter sums to one b1).
            ones_sb = constp.tile([1, 256], BF16, tag="ones")
            nc.gpsimd.memset(ones_sb[:], 1.0)
            b1_sb = constp.tile([1, D_HID], BF16, tag="b1r")
            nc.gpsimd.dma_start(b1_sb[:], b1row[:])
            mm1(ones_sb[:], b1_sb[:], start=True, stop=False)

            at_tiles = {}

            def emit_transpose(k):
                pt = psB.tile([128, 256], BF16, tag="pt")
                at = atp.tile([128, 256], BF16, tag="at")
                for bt in range(2):
                    nc.tensor.transpose(
                        pt[:, bt * 128:(bt + 1) * 128],
                        pcam_tiles[bt][:, k * 128:(k + 1) * 128],
                        id_sb[:],
                    )
                nc.scalar.copy(at[:], pt[:])
                at_tiles[k] = at

            emit_transpose(0)
            emit_transpose(1)
            for k in range(K_TILES):
                if k + 2 < K_TILES:
                    emit_transpose(k + 2)
                w1_t = w1p.tile([128, D_HID], BF16, tag="w1")
                nc.sync.dma_start(w1_t[:], w1t[k * 128:(k + 1) * 128, :])
                at = at_tiles.pop(k)
                mm1(at[:], w1_t[:], start=False, stop=(k == K_TILES - 1))

            # ---- late constants for the epilogue ----
            w2_sb = constp.tile([128, D_TILES * N_CLS], BF16, tag="w2")
            nc.sync.dma_start(
                w2_sb[:].rearrange("p (t n) -> p t n", t=D_TILES),
                w2t[:].rearrange("(t p) n -> p t n", p=128))
            b2_sb = constp.tile([B_SH, N_CLS], F32, tag="b2")
            nc.sync.dma_start(b2_sb[:], b2b[:])

            # ---- stage C: ReduceScatter over the batch dim (bf16) ----
            inb = dramp.tile([B, D_HID], BF16, tag="inb")
            rsb = dramp.tile([B_SH, D_HID], BF16, tag="rsb")
            for bt in range(2):
                h_sb = hsbp.tile([128, D_HID], BF16, tag="hsb",
                                 name=f"hsb{bt}", bufs=2)
                # evacuate the two banks on different engines, in parallel
                nc.scalar.copy(h_sb[:, 0:512], hps[bt][0][:])
                nc.vector.tensor_copy(h_sb[:, 512:768], hps[bt][1][:])
                nc.sync.dma_start(inb[bt * 128:(bt + 1) * 128, :], h_sb[:])
            nc.gpsimd.collective_compute(
                "ReduceScatter",
                mybir.AluOpType.add,
                replica_groups=[list(range(N_CORES))],
                ins=[inb[:].opt()],
                outs=[rsb[:].opt()],
            )

            # ---- stages D+E: transpose the RS shard to d-major, fusing
            # the relu into the PSUM evacuation (relu commutes with
            # transpose; b1 is already folded into h).
            hsh = hsbp.tile([B_SH, D_HID], BF16, tag="hsh")
            nc.sync.dma_start(hsh[:], rsb[:])
            hT2 = hsbp.tile([128, D_TILES * B_SH], BF16, tag="hT2")
            for t in range(D_TILES):
                ptt = psB.tile([128, B_SH], BF16, tag="pt", name=f"ptt{t}")
                nc.tensor.transpose(
                    ptt[:], hsh[:, t * 128:(t + 1) * 128],
                    id_sb[0:B_SH, 0:B_SH])
                nc.scalar.activation(
                    hT2[:, t * B_SH:(t + 1) * B_SH], ptt[:],
                    mybir.ActivationFunctionType.Relu)
            lg = psB.tile([B_SH, N_CLS], F32, tag="pt", name="lgps")
            for t in range(D_TILES):
                nc.tensor.matmul(
                    lg[:],
                    hT2[:, t * B_SH:(t + 1) * B_SH],
                    w2_sb[:, t * N_CLS:(t + 1) * N_CLS],
                    start=(t == 0),
                    stop=(t == D_TILES - 1),
                )
            out_sb = hsbp.tile([B_SH, N_CLS], F32, tag="outsb")
            nc.vector.tensor_add(out_sb[:], lg[:], b2_sb[:])
            nc.sync.dma_start(logits_sh[:], out_sb[:])

    nc.compile()
    return nc


def _prep_inputs(features, W1, b1, W2, b2):
    bf16 = ml_dtypes.bfloat16
    feat_flat = np.asarray(features, np.float32).reshape(B, C, HW)
    W1 = np.asarray(W1, np.float32)
    b1 = np.asarray(b1, np.float32)
    w2t_full = np.ascontiguousarray(
        np.asarray(W2, np.float32).T.astype(bf16))       # [768, 5]
    b2b = np.ascontiguousarray(np.broadcast_to(
        np.asarray(b2, np.float32), (B_SH, N_CLS)))
    ident = np.eye(128, dtype=bf16)
    b1_bf = b1.astype(bf16).reshape(1, D_HID)
    zeros_row = np.zeros((1, D_HID), bf16)

    in_maps = []
    for c in range(N_CORES):
        featN = np.ascontiguousarray(
            feat_flat[:, c * C_PER:(c + 1) * C_PER, :]).reshape(B, F_PER)
        w1t = np.ascontiguousarray(
            W1[:, c * F_PER:(c + 1) * F_PER].T)
        in_maps.append({
            "featN": featN.astype(bf16),
            "w1t": w1t.astype(bf16),
            "b1row": b1_bf if c == 0 else zeros_row,
            "w2t": w2t_full,
            "b2b": b2b,
            "ident": ident,
        })
    return in_maps


def kernel(features, W1, b1, W2, b2):
    if "nc" not in _CACHE:
        _CACHE["nc"] = _build_program()
    nc = _CACHE["nc"]
    in_maps = _prep_inputs(features, W1, b1, W2, b2)
    res = run_bass_kernel_spmd(nc, in_maps, core_ids=list(range(N_CORES)))
    kernel._last_results = res
    logits = np.concatenate(
        [np.asarray(res.results[c]["logits_sh"], np.float32)
         for c in range(N_CORES)], axis=0)
    return (logits, np.asarray(features, np.float32))
